# revision 1
# baseline (speedup 1.0000x reference)
"""COLoRALinear fused kernel for 8 trn2 NeuronCores (Bass/Tile).

Problem: out = x@W.T + b + cw*2*(x@sA.T)@sB.T + (1-cw)*2*sum_t r[b,t]*(x@tA[t].T)@tB[t].T
with routing r = softmax(mean_s(x) @ emb.T), cw = sigmoid(collab_weight).

Sharding: core i -> batch element p=i//2 (2048 tokens), DOUT half h=i%2
(2048 cols). Each core holds its full batch element, so routing is local;
no collectives.

Device plan per core:
  - preload x^T as bf16, SBUF-resident ([128, 32, 2048], 128KB/part)
  - phase A: hid^T[80, 2048] = A_cat @ x^T  (A_cat rows: 8 shared + 64 task
    + 8 task_emb), evict rows 0..71 to bf16, free-reduce rows 72..79 into
    routing logits
  - routing: softmax on one partition, build svec[73] (cw2 / routing-scaled
    / 1.0-for-bias), scale B_cat rows -> bf16
  - main loop: 16 dout-tiles of 128; W^T d-tile streamed fp32->bf16 and
    used as the stationary operand, x^T slices as the moving operand
    (N=512); per d-tile: 4 PSUM banks accumulate 32 k matmuls + 1 LoRA
    down-proj matmul each; evict fp32; store [dout, tok] (host transposes).

Measured (reps-in-NEFF diff): ~408 us/core-iteration, rel err 1.65e-3.
W is staged per k-tile (64KB DMAs) in a dedicated 4-buf pool so the first
d-tiles' matmul chains chase the x preload stream instead of waiting for
it. Deeper W prefetch (wch bufs=3: 574 us) regressed — early W DMAs steal
HBM bandwidth from the startup-critical x preload.
"""
import numpy as np
from contextlib import ExitStack

import concourse.bass as bass
import concourse.tile as tile
from concourse import mybir
from concourse.bass_utils import run_bass_kernel_spmd
from concourse.vector_clock import ScopedClock

B, S, DIN, DOUT, R, T = 4, 2048, 4096, 4096, 8, 8
SCALING = 2.0
N_CORES = 8
P = 128
KT = DIN // P            # 32 k-tiles
S_CORE = S               # tokens per core (one batch element)
N_CORE = DOUT // 2       # dout columns per core
NCH = 256                # n-chunk width
NNC = N_CORE // NCH      # 8 n-chunks
MT = S_CORE // P         # 16 m-tiles
AROWS = 80               # 8 shared + 64 task + 8 emb rows in A_cat
HID = 73                 # 72 lora rows + ones(bias) row
F32 = mybir.dt.float32
BF16 = mybir.dt.bfloat16


class _DrainSplitTileContext(tile.TileContext):
    """Walrus in this container rejects a Drain carrying >1 sem wait (the
    CTRL_NO encoding has one TPB_EVENTS wait slot). Split the exit drain's
    waits across a chain of single-wait drains."""

    def _drain_and_barrier(self, tick_clock, wait_clock):
        drain_inst = self.nc.sync.drain()
        wait_clock.add_sem_waits(
            drain_inst.ins, ScopedClock({None: tick_clock.global_clock})
        )
        si = drain_inst.ins.sync_info
        if si is not None and len(si.on_wait) > 1:
            waits = list(si.on_wait)
            drain_inst.ins.sync_info = mybir.SyncInfo(
                on_wait=[waits[0]], on_update=list(si.on_update)
            )
            for w in waits[1:]:
                extra = self.nc.sync.drain()
                extra.ins.sync_info = mybir.SyncInfo(on_wait=[w], on_update=[])

        self.nc.all_engine_barrier()
        assert self.sems is not None
        popped = self.nc._tile_sem_poison_stack.pop()
        assert popped is self._sem_poison
        self.nc.clear_and_free_semaphores(list(self.sems.allocated().values()))
        self.nc.all_engine_barrier()


_wsplit_counter = [0]


def _split_multi_waits(nc):
    """Walrus here lowers DMA/CTRL instructions with a single TPB_EVENTS wait
    slot and rejects >1 sem wait. Hoist extra waits onto same-engine NoOps
    inserted immediately before the offending instruction (engine program
    order makes this semantics-preserving)."""
    for f in nc.m.functions:
        for blk in f.blocks:
            insts = blk.instructions
            out = []
            changed = False
            for inst in insts:
                si = inst.sync_info
                if si is not None and len(si.on_wait) > 1:
                    waits = list(si.on_wait)
                    for w in waits[:-1]:
                        _wsplit_counter[0] += 1
                        nop = mybir.InstNoOp(name=f"I-wsplit-{_wsplit_counter[0]}")
                        nop.engine = inst.engine
                        nop.sync_info = mybir.SyncInfo(on_wait=[w], on_update=[])
                        out.append(nop)
                    inst.sync_info = mybir.SyncInfo(
                        on_wait=[waits[-1]], on_update=list(si.on_update)
                    )
                    changed = True
                out.append(inst)
            if changed:
                blk.instructions = out


def build_nc(reps: int = 1):
    nc = bass.Bass(trn_type="TRN2", target_bir_lowering=False)
    xt = nc.dram_tensor("xt", [DIN, S_CORE], F32, kind="ExternalInput").ap()
    wt = nc.dram_tensor("wt", [DIN, N_CORE], F32, kind="ExternalInput").ap()
    act = nc.dram_tensor("act", [KT, P, AROWS], F32, kind="ExternalInput").ap()
    bcat = nc.dram_tensor("bcat", [HID, N_CORE], F32, kind="ExternalInput").ap()
    cw = nc.dram_tensor("cw", [1, 1], F32, kind="ExternalInput").ap()
    # output stored [dout, tok]; host assembly transposes back
    out = nc.dram_tensor("out", [N_CORE, S_CORE], F32, kind="ExternalOutput").ap()

    xt_r = xt.rearrange("(kt p) t -> p kt t", p=P)
    wt_r = wt.rearrange("(kt p) n -> p kt n", p=P)

    with _DrainSplitTileContext(nc) as tc, ExitStack() as ctx:
        xres_p = ctx.enter_context(tc.tile_pool(name="xres", bufs=1))
        wch_p = ctx.enter_context(tc.tile_pool(name="wch", bufs=2))
        stage_p = ctx.enter_context(tc.tile_pool(name="stage", bufs=2))
        wstage_p = ctx.enter_context(tc.tile_pool(name="wstage", bufs=4))
        abf_p = ctx.enter_context(tc.tile_pool(name="abf", bufs=1))
        small_p = ctx.enter_context(tc.tile_pool(name="small", bufs=1))
        evict_p = ctx.enter_context(tc.tile_pool(name="evict", bufs=3))
        psb_p = ctx.enter_context(tc.tile_pool(name="psb", bufs=7, space="PSUM"))
        pss_p = ctx.enter_context(tc.tile_pool(name="pss", bufs=1, space="PSUM"))

        for _rep in range(reps):
            # ---- constants / small preloads ----
            a_bf = abf_p.tile([P, KT, AROWS], BF16)
            act_r = act.rearrange("kt p c -> p kt c")
            for half in range(2):
                a_st = stage_p.tile([P, KT // 2, AROWS], F32, tag="stage")
                ks = slice(half * KT // 2, (half + 1) * KT // 2)
                nc.sync.dma_start(out=a_st[:], in_=act_r[:, ks, :])
                nc.vector.tensor_copy(out=a_bf[:, ks, :], in_=a_st[:])

            bmat = small_p.tile([HID, N_CORE], F32)
            nc.sync.dma_start(out=bmat[:], in_=bcat)

            cwt = small_p.tile([1, 1], F32)
            nc.sync.dma_start(out=cwt[:], in_=cw)
            sig = small_p.tile([1, 1], F32)
            nc.scalar.activation(
                out=sig[:], in_=cwt[:], func=mybir.ActivationFunctionType.Sigmoid
            )
            cw2 = small_p.tile([1, 1], F32)
            nc.vector.tensor_scalar_mul(cw2[:], sig[:], SCALING)
            tsc = small_p.tile([1, 1], F32)  # (1 - sigmoid) * SCALING
            nc.vector.tensor_scalar(
                out=tsc[:], in0=sig[:], scalar1=-SCALING, scalar2=SCALING,
                op0=mybir.AluOpType.mult, op1=mybir.AluOpType.add,
            )

            # ---- x preload (fp32 -> bf16, SBUF resident) ----
            xres = xres_p.tile([P, KT, S_CORE], BF16)
            for kt in range(KT):
                xs = stage_p.tile([P, S_CORE], F32, tag="stage")
                nc.sync.dma_start(out=xs[:], in_=xt_r[:, kt, :])
                nc.vector.tensor_copy(out=xres[:, kt, :], in_=xs[:])

            # ---- phase A: hid^T = A_cat @ x^T ----
            hid = small_p.tile([HID, S_CORE], BF16)
            hacc = small_p.tile([AROWS, 1], F32)     # free-reduced phase-A rows
            hpart = small_p.tile([AROWS, 4], F32)
            for c in range(4):
                ph = pss_p.tile([AROWS, 512], F32, tag="pss")
                for kt in range(KT):
                    nc.tensor.matmul(
                        ph[:], lhsT=a_bf[:, kt, :], rhs=xres[:, kt, c * 512:(c + 1) * 512],
                        start=(kt == 0), stop=(kt == KT - 1),
                    )
                nc.vector.tensor_copy(out=hid[0:72, c * 512:(c + 1) * 512], in_=ph[0:72, :])
                nc.vector.tensor_reduce(
                    out=hpart[:, c:c + 1], in_=ph[:], axis=mybir.AxisListType.X,
                    op=mybir.AluOpType.add,
                )
            ones_s = small_p.tile([1, P], BF16)
            nc.vector.memset(ones_s[:], 1.0)
            for m in range(MT):
                nc.sync.dma_start(out=hid[72:73, m * P:(m + 1) * P], in_=ones_s[:])
            nc.vector.tensor_reduce(
                out=hacc[:], in_=hpart[:], axis=mybir.AxisListType.X,
                op=mybir.AluOpType.add,
            )

            # ---- routing ----
            l_row = small_p.tile([1, 8], F32)
            nc.sync.dma_start(out=l_row[:], in_=hacc[72:80, 0:1])  # partition->free
            e_row = small_p.tile([1, 8], F32)
            nc.scalar.activation(
                out=e_row[:], in_=l_row[:], func=mybir.ActivationFunctionType.Exp,
                scale=1.0 / S,
            )
            ssum = small_p.tile([1, 1], F32)
            nc.vector.tensor_reduce(
                out=ssum[:], in_=e_row[:], axis=mybir.AxisListType.X,
                op=mybir.AluOpType.add,
            )
            rec = small_p.tile([1, 1], F32)
            nc.vector.reciprocal(out=rec[:], in_=ssum[:])
            comb = small_p.tile([1, 1], F32)  # (1/sum) * (1-cw)*SCALING
            nc.vector.tensor_tensor(
                out=comb[:], in0=rec[:], in1=tsc[:], op=mybir.AluOpType.mult
            )
            ones8 = small_p.tile([1, 8], F32)
            nc.vector.memset(ones8[:], 1.0)
            svec_f = small_p.tile([1, HID], F32)
            nc.vector.tensor_scalar(
                out=svec_f[0:1, 0:8], in0=ones8[:], scalar1=cw2[:], scalar2=None,
                op0=mybir.AluOpType.mult,
            )
            for t in range(T):
                nc.vector.tensor_scalar(
                    out=svec_f[0:1, 8 + 8 * t:16 + 8 * t], in0=ones8[:],
                    scalar1=e_row[0:1, t:t + 1], scalar2=comb[:],
                    op0=mybir.AluOpType.mult, op1=mybir.AluOpType.mult,
                )
            nc.vector.memset(svec_f[0:1, 72:73], 1.0)
            svec = small_p.tile([HID, 1], F32)
            nc.sync.dma_start(out=svec[:], in_=svec_f[:])  # free->partition
            bbf = small_p.tile([HID, N_CORE], BF16)
            nc.vector.tensor_scalar(
                out=bbf[:], in0=bmat[:], scalar1=svec[:], scalar2=None,
                op0=mybir.AluOpType.mult,
            )

            # ---- main loop: base matmul + fused down-proj ----
            # W^T d-tile is the stationary operand, x^T the moving one
            # (N=512); PSUM tiles come out [dout, tok].
            TC = 4  # token chunks of 512
            for d in range(N_CORE // P):
                wch = wch_p.tile([P, KT, P], BF16)
                for kt in range(KT):
                    ws = wstage_p.tile([P, P], F32)
                    nc.sync.dma_start(
                        out=ws[:], in_=wt_r[:, kt, d * P:(d + 1) * P]
                    )
                    nc.vector.tensor_copy(out=wch[:, kt, :], in_=ws[:])
                pss = [
                    psb_p.tile([P, 512], F32, tag="ps", name=f"ps_{_rep}_{d}_{i}")
                    for i in range(TC)
                ]
                for kt in range(KT):
                    for tcI in range(TC):
                        nc.tensor.matmul(
                            pss[tcI][:], lhsT=wch[:, kt, :],
                            rhs=xres[:, kt, tcI * 512:(tcI + 1) * 512],
                            start=(kt == 0), stop=False,
                        )
                for tcI in range(TC):
                    nc.tensor.matmul(
                        pss[tcI][:], lhsT=bbf[:, d * P:(d + 1) * P],
                        rhs=hid[:, tcI * 512:(tcI + 1) * 512],
                        start=False, stop=True,
                    )
                    ev = evict_p.tile([P, 512], F32)
                    nc.scalar.activation(
                        out=ev[:], in_=pss[tcI][:],
                        func=mybir.ActivationFunctionType.Copy,
                    )
                    nc.scalar.dma_start(
                        out=out[d * P:(d + 1) * P, tcI * 512:(tcI + 1) * 512],
                        in_=ev[:],
                    )
    _split_multi_waits(nc)
    return nc


def prep_inputs(x, W, b, shared_A, shared_B, task_A, task_B, task_emb, collab_weight):
    """Host-side sharding/layout prep. Pure layout: slice/transpose/concat."""
    x = np.asarray(x, dtype=np.float32)
    W = np.asarray(W, dtype=np.float32)
    b = np.asarray(b, dtype=np.float32)
    a_cat = np.concatenate(
        [np.asarray(shared_A), np.asarray(task_A).reshape(T * R, DIN),
         np.asarray(task_emb)], axis=0
    ).astype(np.float32)                                   # [80, DIN]
    act = np.ascontiguousarray(a_cat.T.reshape(KT, P, AROWS))
    cwv = np.asarray(collab_weight, dtype=np.float32).reshape(1, 1)

    xt = [np.ascontiguousarray(x[p].T) for p in range(B)]  # [DIN, S] each
    wt, bc = [], []
    for h in range(2):
        cols = slice(h * N_CORE, (h + 1) * N_CORE)
        wt.append(np.ascontiguousarray(W[cols, :].T))      # [DIN, N_CORE]
        bcat = np.empty((HID, N_CORE), dtype=np.float32)
        bcat[0:8] = np.asarray(shared_B)[cols, :].T
        bcat[8:72] = np.asarray(task_B)[:, cols, :].transpose(0, 2, 1).reshape(
            T * R, N_CORE
        )
        bcat[72] = b[cols]
        bc.append(bcat)

    in_maps = []
    for i in range(N_CORES):
        p, h = i // 2, i % 2
        in_maps.append(
            {"xt": xt[p], "wt": wt[h], "act": act, "bcat": bc[h], "cw": cwv}
        )
    return in_maps


def assemble(results):
    out = np.empty((B, S, DOUT), dtype=np.float32)
    for i in range(N_CORES):
        p, h = i // 2, i % 2
        out[p, :, h * N_CORE:(h + 1) * N_CORE] = results[i]["out"].T
    return out


_NC_CACHE = None


def kernel(**inputs) -> np.ndarray:
    global _NC_CACHE
    if _NC_CACHE is None:
        _NC_CACHE = build_nc()
    in_maps = prep_inputs(**inputs)
    res = run_bass_kernel_spmd(_NC_CACHE, in_maps, core_ids=list(range(N_CORES)))
    return assemble(res.results)



# revision 6
# speedup vs baseline: 1.1700x; 1.1700x over previous
"""COLoRALinear fused kernel for 8 trn2 NeuronCores (Bass/Tile).

Problem: out = x@W.T + b + cw*2*(x@sA.T)@sB.T + (1-cw)*2*sum_t r[b,t]*(x@tA[t].T)@tB[t].T
with routing r = softmax(mean_s(x) @ emb.T), cw = sigmoid(collab_weight).

Sharding: core i -> batch element p=i//2 (2048 tokens), DOUT half h=i%2
(2048 cols). Each core holds its full batch element, so routing is local;
no collectives.

v2 design (PE-roofline focused; PE stream is ~497us/rep of matmuls):
  - All inputs host-converted to bf16 -> direct DMA into SBUF, no f32
    staging or on-device conversion (halves HBM traffic, frees DVE).
  - x lives as 32 per-k-tile tiles ([128,2048] bf16) in a 36-deep ring:
    next rep's preload streams in under the current rep's main-loop tail
    instead of serializing ~88us at the rep boundary.
  - Phase A (hid^T = A_cat @ x^T) is kt-outer across 4 token chunks so
    the last k-tiles are only needed at the END of phase A, tolerating
    the preload tail.
  - PSUM: 4 chunk-tags x 2 bufs x [128,512] f32 = all 8 banks; phase A
    borrows one buf of each tag. Main d-tile accumulates 32 k matmuls +
    1 LoRA down-proj per chunk, evicts bf16 (ACT/DVE alternating), one
    [128, 4KB] DMA out per d-tile.
  - W d-tile is one contiguous [128, 8KB] bf16 DMA (host pre-tiled),
    double-buffered on the gpsimd queue.
Output is [dout, tok] bf16; host upconverts + transposes.
"""
import numpy as np
import ml_dtypes
from contextlib import ExitStack

import concourse.bass as bass
import concourse.tile as tile
from concourse import mybir
from concourse.bass_utils import run_bass_kernel_spmd
from concourse.vector_clock import ScopedClock

B, S, DIN, DOUT, R, T = 4, 2048, 4096, 4096, 8, 8
SCALING = 2.0
N_CORES = 8
P = 128
KT = DIN // P            # 32 k-tiles
S_CORE = S               # tokens per core (one batch element)
N_CORE = DOUT // 2       # dout columns per core
ND = N_CORE // P         # 16 d-tiles
NCH = 512                # psum chunk width (1 bank)
NC4 = S_CORE // NCH      # 4 chunks
AROWS = 80               # 8 shared + 64 task + 8 emb rows in A_cat
HID = 73                 # 72 lora rows + ones(bias) row
XBUFS = 36               # x ring depth (KT in flight + 4 for cross-rep overlap)
F32 = mybir.dt.float32
BF16 = mybir.dt.bfloat16
BF16NP = ml_dtypes.bfloat16


class _DrainSplitTileContext(tile.TileContext):
    """Walrus in this container rejects a Drain carrying >1 sem wait (the
    CTRL_NO encoding has one TPB_EVENTS wait slot). Split the exit drain's
    waits across a chain of single-wait drains."""

    def _drain_and_barrier(self, tick_clock, wait_clock):
        drain_inst = self.nc.sync.drain()
        wait_clock.add_sem_waits(
            drain_inst.ins, ScopedClock({None: tick_clock.global_clock})
        )
        si = drain_inst.ins.sync_info
        if si is not None and len(si.on_wait) > 1:
            waits = list(si.on_wait)
            drain_inst.ins.sync_info = mybir.SyncInfo(
                on_wait=[waits[0]], on_update=list(si.on_update)
            )
            for w in waits[1:]:
                extra = self.nc.sync.drain()
                extra.ins.sync_info = mybir.SyncInfo(on_wait=[w], on_update=[])

        self.nc.all_engine_barrier()
        assert self.sems is not None
        popped = self.nc._tile_sem_poison_stack.pop()
        assert popped is self._sem_poison
        self.nc.clear_and_free_semaphores(list(self.sems.allocated().values()))
        self.nc.all_engine_barrier()


_wsplit_counter = [0]


def _split_multi_waits(nc):
    """Walrus here lowers DMA/CTRL instructions with a single TPB_EVENTS wait
    slot and rejects >1 sem wait. Hoist extra waits onto same-engine NoOps
    inserted immediately before the offending instruction (engine program
    order makes this semantics-preserving)."""
    for f in nc.m.functions:
        for blk in f.blocks:
            insts = blk.instructions
            out = []
            changed = False
            for inst in insts:
                si = inst.sync_info
                if si is not None and len(si.on_wait) > 1:
                    waits = list(si.on_wait)
                    for w in waits[:-1]:
                        _wsplit_counter[0] += 1
                        nop = mybir.InstNoOp(name=f"I-wsplit-{_wsplit_counter[0]}")
                        nop.engine = inst.engine
                        nop.sync_info = mybir.SyncInfo(on_wait=[w], on_update=[])
                        out.append(nop)
                    inst.sync_info = mybir.SyncInfo(
                        on_wait=[waits[-1]], on_update=list(si.on_update)
                    )
                    changed = True
                out.append(inst)
            if changed:
                blk.instructions = out


def build_nc(reps: int = 1):
    nc = bass.Bass(trn_type="TRN2", target_bir_lowering=False)
    xt = nc.dram_tensor("xt", [DIN, S_CORE], BF16, kind="ExternalInput").ap()
    wt = nc.dram_tensor("wt", [ND, P, KT * P], BF16, kind="ExternalInput").ap()
    act = nc.dram_tensor("act", [P, KT, AROWS], BF16, kind="ExternalInput").ap()
    bcat = nc.dram_tensor("bcat", [HID, N_CORE], BF16, kind="ExternalInput").ap()
    cw = nc.dram_tensor("cw", [1, 1], F32, kind="ExternalInput").ap()
    # output stored [dout, tok] bf16; host upconverts + transposes
    out = nc.dram_tensor("out", [N_CORE, S_CORE], BF16, kind="ExternalOutput").ap()

    xt_r = xt.rearrange("(kt p) t -> p kt t", p=P)
    out_r = out.rearrange("(d p) t -> p d t", p=P)

    with _DrainSplitTileContext(nc) as tc, ExitStack() as ctx:
        xp = ctx.enter_context(tc.tile_pool(name="xp", bufs=XBUFS))
        wch_p = ctx.enter_context(tc.tile_pool(name="wch", bufs=2))
        abf_p = ctx.enter_context(tc.tile_pool(name="abf", bufs=2))
        bmat_p = ctx.enter_context(tc.tile_pool(name="bmat", bufs=2))
        small_p = ctx.enter_context(tc.tile_pool(name="small", bufs=2))
        pers_p = ctx.enter_context(tc.tile_pool(name="pers", bufs=1))
        evict_p = ctx.enter_context(tc.tile_pool(name="ev", bufs=2))
        ps_p = ctx.enter_context(tc.tile_pool(name="ps", bufs=2, space="PSUM"))

        # persistent across reps: ones(bias) row of hid; bbf rewritten per rep
        hid = pers_p.tile([HID, S_CORE], BF16)
        ones_row = pers_p.tile([1, S_CORE], BF16)
        nc.vector.memset(ones_row[:], 1.0)
        nc.sync.dma_start(out=hid[72:73, :], in_=ones_row[:])
        bbf = pers_p.tile([HID, N_CORE], BF16)

        for _rep in range(reps):
            # ---- small loads (vector queue; must not sit behind x ring) ----
            a_bf = abf_p.tile([P, KT, AROWS], BF16, tag="abf", name=f"a_bf_{_rep}")
            nc.gpsimd.dma_start(out=a_bf[:], in_=act)
            bmat = bmat_p.tile([HID, N_CORE], BF16, tag="bmat", name=f"bmat_{_rep}")
            nc.gpsimd.dma_start(out=bmat[:], in_=bcat)
            cwt = small_p.tile([1, 1], F32, tag="cwt", name=f"cwt_{_rep}")
            nc.scalar.dma_start(out=cwt[:], in_=cw)
            sig = small_p.tile([1, 1], F32, tag="sig", name=f"sig_{_rep}")
            nc.scalar.activation(
                out=sig[:], in_=cwt[:], func=mybir.ActivationFunctionType.Sigmoid
            )
            cw2 = small_p.tile([1, 1], F32, tag="cw2", name=f"cw2_{_rep}")
            nc.vector.tensor_scalar_mul(cw2[:], sig[:], SCALING)
            tsc = small_p.tile([1, 1], F32, tag="tsc", name=f"tsc_{_rep}")
            nc.vector.tensor_scalar(
                out=tsc[:], in0=sig[:], scalar1=-SCALING, scalar2=SCALING,
                op0=mybir.AluOpType.mult, op1=mybir.AluOpType.add,
            )

            # ---- x preload into the ring (sync queue) ----
            xs = []
            for kt in range(KT):
                xk = xp.tile([P, S_CORE], BF16, tag="xk", name=f"xk_{_rep}_{kt}")
                nc.sync.dma_start(out=xk[:], in_=xt_r[:, kt, :])
                xs.append(xk)

            # ---- phase A: hid^T = A_cat @ x^T, kt-outer over 4 chunks ----
            pa = [
                ps_p.tile([AROWS, NCH], F32, tag=f"ps{c}", name=f"pa{c}_{_rep}")
                for c in range(NC4)
            ]
            for kt in range(KT):
                for c in range(NC4):
                    nc.tensor.matmul(
                        pa[c][:], lhsT=a_bf[:, kt, :],
                        rhs=xs[kt][:, c * NCH:(c + 1) * NCH],
                        start=(kt == 0), stop=(kt == KT - 1),
                    )
            # evict lora rows to bf16 hid; free-reduce routing rows
            hpart = small_p.tile([AROWS, NC4], F32, tag="hpart", name=f"hpart_{_rep}")
            for c in range(NC4):
                nc.vector.tensor_copy(
                    out=hid[0:72, c * NCH:(c + 1) * NCH], in_=pa[c][0:72, :]
                )
                nc.vector.tensor_reduce(
                    out=hpart[:, c:c + 1], in_=pa[c][:],
                    axis=mybir.AxisListType.X, op=mybir.AluOpType.add,
                )
            hacc = small_p.tile([AROWS, 1], F32, tag="hacc", name=f"hacc_{_rep}")
            nc.vector.tensor_reduce(
                out=hacc[:], in_=hpart[:], axis=mybir.AxisListType.X,
                op=mybir.AluOpType.add,
            )

            # ---- routing ----
            l_row = small_p.tile([1, 8], F32, tag="l_row", name=f"l_row_{_rep}")
            nc.sync.dma_start(out=l_row[:], in_=hacc[72:80, 0:1])  # partition->free
            e_row = small_p.tile([1, 8], F32, tag="e_row", name=f"e_row_{_rep}")
            nc.scalar.activation(
                out=e_row[:], in_=l_row[:], func=mybir.ActivationFunctionType.Exp,
                scale=1.0 / S,
            )
            ssum = small_p.tile([1, 1], F32, tag="ssum", name=f"ssum_{_rep}")
            nc.vector.tensor_reduce(
                out=ssum[:], in_=e_row[:], axis=mybir.AxisListType.X,
                op=mybir.AluOpType.add,
            )
            rec = small_p.tile([1, 1], F32, tag="rec", name=f"rec_{_rep}")
            nc.vector.reciprocal(out=rec[:], in_=ssum[:])
            comb = small_p.tile([1, 1], F32, tag="comb", name=f"comb_{_rep}")
            nc.vector.tensor_tensor(
                out=comb[:], in0=rec[:], in1=tsc[:], op=mybir.AluOpType.mult
            )
            ones8 = small_p.tile([1, 8], F32, tag="ones8", name=f"ones8_{_rep}")
            nc.vector.memset(ones8[:], 1.0)
            svec_f = small_p.tile([1, HID], F32, tag="svec_f", name=f"svec_f_{_rep}")
            nc.vector.tensor_scalar(
                out=svec_f[0:1, 0:8], in0=ones8[:], scalar1=cw2[:], scalar2=None,
                op0=mybir.AluOpType.mult,
            )
            for t in range(T):
                nc.vector.tensor_scalar(
                    out=svec_f[0:1, 8 + 8 * t:16 + 8 * t], in0=ones8[:],
                    scalar1=e_row[0:1, t:t + 1], scalar2=comb[:],
                    op0=mybir.AluOpType.mult, op1=mybir.AluOpType.mult,
                )
            nc.vector.memset(svec_f[0:1, 72:73], 1.0)
            svec = small_p.tile([HID, 1], F32, tag="svec", name=f"svec_{_rep}")
            nc.sync.dma_start(out=svec[:], in_=svec_f[:])  # free->partition
            nc.vector.tensor_scalar(
                out=bbf[:], in0=bmat[:], scalar1=svec[:], scalar2=None,
                op0=mybir.AluOpType.mult,
            )

            # ---- main loop: base matmul + fused down-proj ----
            for d in range(ND):
                wch = wch_p.tile([P, KT * P], BF16, tag="wch", name=f"wch_{_rep}_{d}")
                nc.gpsimd.dma_start(out=wch[:], in_=wt[d])
                pss = [
                    ps_p.tile([P, NCH], F32, tag=f"ps{c}", name=f"ps{c}_{_rep}_{d}")
                    for c in range(NC4)
                ]
                for kt in range(KT):
                    for c in range(NC4):
                        nc.tensor.matmul(
                            pss[c][:], lhsT=wch[:, kt * P:(kt + 1) * P],
                            rhs=xs[kt][:, c * NCH:(c + 1) * NCH],
                            start=(kt == 0), stop=False,
                        )
                ev = evict_p.tile([P, S_CORE], BF16, tag="ev", name=f"ev_{_rep}_{d}")
                for c in range(NC4):
                    nc.tensor.matmul(
                        pss[c][:], lhsT=bbf[:, d * P:(d + 1) * P],
                        rhs=hid[:, c * NCH:(c + 1) * NCH],
                        start=False, stop=True,
                    )
                    if c % 2 == 0:
                        nc.scalar.activation(
                            out=ev[:, c * NCH:(c + 1) * NCH], in_=pss[c][:],
                            func=mybir.ActivationFunctionType.Copy,
                        )
                    else:
                        nc.vector.tensor_copy(
                            out=ev[:, c * NCH:(c + 1) * NCH], in_=pss[c][:]
                        )
                nc.scalar.dma_start(out=out_r[:, d, :], in_=ev[:])
    _split_multi_waits(nc)
    return nc


def prep_inputs(x, W, b, shared_A, shared_B, task_A, task_B, task_emb, collab_weight):
    """Host-side sharding/layout prep: slice/transpose/concat + bf16 cast."""
    x = np.asarray(x, dtype=np.float32)
    W = np.asarray(W, dtype=np.float32)
    b = np.asarray(b, dtype=np.float32)
    a_cat = np.concatenate(
        [np.asarray(shared_A), np.asarray(task_A).reshape(T * R, DIN),
         np.asarray(task_emb)], axis=0
    ).astype(np.float32)                                   # [80, DIN]
    # [P, KT, AROWS]: partition p, k-tile kt, row r  <-  a_cat[r, kt*P+p]
    act = np.ascontiguousarray(
        a_cat.T.reshape(KT, P, AROWS).transpose(1, 0, 2)
    ).astype(BF16NP)
    cwv = np.asarray(collab_weight, dtype=np.float32).reshape(1, 1)

    xt = [np.ascontiguousarray(x[p].T).astype(BF16NP) for p in range(B)]
    wt, bc = [], []
    for h in range(2):
        cols = slice(h * N_CORE, (h + 1) * N_CORE)
        # [ND, P, KT*P]: d-tile d, partition p(k-lane), (kt, c) flat
        whalf = W[cols, :].T                               # [DIN, N_CORE]
        wtile = (
            whalf.reshape(KT, P, ND, P)                    # [kt, p, d, c]
            .transpose(2, 1, 0, 3)                         # [d, p, kt, c]
            .reshape(ND, P, KT * P)
        )
        wt.append(np.ascontiguousarray(wtile).astype(BF16NP))
        bcat = np.empty((HID, N_CORE), dtype=np.float32)
        bcat[0:8] = np.asarray(shared_B)[cols, :].T
        bcat[8:72] = np.asarray(task_B)[:, cols, :].transpose(0, 2, 1).reshape(
            T * R, N_CORE
        )
        bcat[72] = b[cols]
        bc.append(bcat.astype(BF16NP))

    in_maps = []
    for i in range(N_CORES):
        p, h = i // 2, i % 2
        in_maps.append(
            {"xt": xt[p], "wt": wt[h], "act": act, "bcat": bc[h], "cw": cwv}
        )
    return in_maps


def assemble(results):
    out = np.empty((B, S, DOUT), dtype=np.float32)
    for i in range(N_CORES):
        p, h = i // 2, i % 2
        out[p, :, h * N_CORE:(h + 1) * N_CORE] = (
            results[i]["out"].astype(np.float32).T
        )
    return out


_NC_CACHE = None


def kernel(**inputs) -> np.ndarray:
    global _NC_CACHE
    if _NC_CACHE is None:
        _NC_CACHE = build_nc()
    in_maps = prep_inputs(**inputs)
    res = run_bass_kernel_spmd(_NC_CACHE, in_maps, core_ids=list(range(N_CORES)))
    return assemble(res.results)
